# revision 1
# baseline (speedup 1.0000x reference)
"""Trainium2 Bass kernel for nn_AttentionLayer (B=2, L=S=2048, D=1024, H=16).

Sharding: batch x head-group. Core c handles batch b=c//4 and heads
[4*(c%4), 4*(c%4)+4). Column-parallel Wq/Wk/Wv, row-parallel We; the
per-core partial outputs are summed on the host (4 partials per batch).

vs original baseline:
  - all matmul operands bf16 (host converts inputs/weights): halves the
    input DMA stream (24MB -> 12MB per core) and PE power.
  - phase 1 Q/K projections run weight-stationary (loop dt outer,
    l-chunks inner, 4 PSUM accumulators) so each LDWEIGHTS serves 4
    matmuls instead of 1.
  - output projection stacks head PAIRS on 128 partitions (contraction
    128 instead of 64): half the matmuls. Odd heads' normalized outT is
    moved to partitions 64-127 with an SBUF->SBUF DMA (engines cannot
    shift partition bases; DMA can).
  - softmax normalize: DVE reciprocal on the denominator row, then a
    DRAM-bounce DMA broadcasts it across 64 partitions (off-engine), and
    av_bufs=3 hides the whole chain behind the next head-pair's matmuls.
    (reciprocal_approx_fast crashes the exec unit on this runtime and
    gpsimd partition_broadcast returns garbage -- both avoided.)
  - normalize/outT-shift DMAs ride the idle gpsimd queue; input weight
    DMAs are emitted just-in-time so the PE starts after wq + chunk0.

Host folds the zero-cost pieces: bv and be shift every output row by
(bv @ We + be) because softmax rows sum to 1; bq/bk are applied
on-device (per-partition scalar adds).
"""
import sys

for _p in ("/opt/trn_rl_repo", "/root/.axon_site/_ro/trn_rl_repo"):
    if _p not in sys.path:
        sys.path.insert(0, _p)

import numpy as np

import concourse.bass as bass
import concourse.mybir as mybir
from concourse import bacc
from concourse.bass import AP
from concourse.tile import TileContext

F32 = mybir.dt.float32
F32R = mybir.dt.float32r
BF16 = mybir.dt.bfloat16

D = 1024          # model dim
H_TOTAL = 16
HG = 4            # heads per core
E = 64            # head dim
M = HG * E        # 256 projected cols per core
DT = D // 128     # 8 d-tiles
LCH = 512         # l-chunk
B = 2
N_CORES = 8


def build_program(L=2048, S=2048, sg=2, ex_bufs=3, stream_bufs=8, ob_bufs=3,
                  o_bufs=2, n_bufs=3, av_bufs=3, fin_bufs=1, p1_bufs=8,
                  norm_mode="bounce", debug=0, recip="exact", pair_out=1,
                  p2dt="bf16"):
    nc = bacc.Bacc("TRN2")
    PDT = BF16 if p2dt == "bf16" else F32R
    QT = nc.dram_tensor("QT", [D, L], BF16, kind="ExternalInput")
    KT = nc.dram_tensor("KT", [D, S], BF16, kind="ExternalInput")
    VT = nc.dram_tensor("VT", [D, S], BF16, kind="ExternalInput")
    WQ = nc.dram_tensor("WQ", [D, M], BF16, kind="ExternalInput")
    WK = nc.dram_tensor("WK", [D, M], BF16, kind="ExternalInput")
    WV = nc.dram_tensor("WV", [D, M], BF16, kind="ExternalInput")
    WEDT = BF16 if p2dt == "bf16" else F32R
    WE = nc.dram_tensor("WE", [M, D], WEDT, kind="ExternalInput")
    BQ = nc.dram_tensor("BQ", [2, 128], F32, kind="ExternalInput")
    BK = nc.dram_tensor("BK", [2, 128], F32, kind="ExternalInput")
    OUT = nc.dram_tensor("OUT", [L, D], F32, kind="ExternalOutput")
    RB = (nc.dram_tensor("RB", [(L // LCH) * HG, LCH], F32, kind="Internal")
          if norm_mode == "bounce" else None)

    n_lch = L // LCH
    n_st = S // 128
    n_sg = n_st // sg
    EXP = mybir.ActivationFunctionType.Exp

    with TileContext(nc) as tc:
        with tc.tile_pool(name="const", bufs=1) as cpool, \
             tc.tile_pool(name="stream", bufs=stream_bufs) as spool, \
             tc.tile_pool(name="ex", bufs=ex_bufs) as expool, \
             tc.tile_pool(name="norm", bufs=n_bufs) as npool, \
             tc.tile_pool(name="outw", bufs=o_bufs) as opool, \
             tc.tile_pool(name="ob", bufs=ob_bufs) as obpool:

            wq_sb = cpool.tile([128, DT, M], BF16, tag="wq")
            wk_sb = cpool.tile([128, DT, M], BF16, tag="wk")
            wv_sb = cpool.tile([128, DT, M], BF16, tag="wv")
            if pair_out:
                we_sb = cpool.tile([128, 2, D], WEDT, tag="we")
                we_re = WE.rearrange("(h e) n -> e h n", e=128)
            else:
                we_sb = cpool.tile([64, HG, D], WEDT, tag="we")
                we_re = WE.rearrange("(h e) n -> e h n", e=64)
            bq_sb = cpool.tile([128, 2], F32, tag="bq")
            bk_sb = cpool.tile([128, 2], F32, tag="bk")
            nc.sync.dma_start(wq_sb[:, :, :], WQ.rearrange("(t p) m -> p t m", p=128))
            nc.sync.dma_start(bq_sb[:, :], BQ.rearrange("t p -> p t"))

            qT_sb = cpool.tile([128, 2, L], PDT, tag="qT")
            kT_sb = cpool.tile([128, 2, S], PDT, tag="kT")
            v_sb = cpool.tile([128, n_st, HG * 65], PDT, tag="v")
            # ones-column per head (denominator trick): pre-fill everything
            # with 1.0; the projection copies overwrite the 64 value columns
            # of each head, leaving column 64 as the ones column.
            if p2dt == "bf16":
                nc.vector.memset(v_sb[:, :, :], 1.0)
            else:
                nc.vector.memset(v_sb[:, :, :].bitcast(F32), 1.0)

            # ---- phase 1: projections ----
            def emit_qk_proj(XT, w_sb, b_sb, dst_fn, nch, p1pool,
                             first=False):
                # weight-stationary: all chunks resident, loop (mt, dt)
                # outer and chunks inner so each stationary is loaded once
                # per nch moving matmuls.
                xtr = XT.rearrange("(t p) l -> p t l", p=128)
                chs = []
                for lc in range(nch):
                    ch = spool.tile([128, DT, LCH], BF16, tag="ch",
                                    name=f"ch_{lc}")
                    nc.sync.dma_start(
                        ch[:, :, :], xtr[:, :, lc * LCH:(lc + 1) * LCH]
                    )
                    chs.append(ch)
                for mt in range(2):
                    pss = [p1pool.tile([128, LCH], F32, tag="p1",
                                       name=f"ps_{mt}_{lc}")
                           for lc in range(nch)]
                    for dt in range(DT):
                        for lc in range(nch):
                            nc.tensor.matmul(
                                pss[lc][:, :],
                                w_sb[:, dt, mt * 128:(mt + 1) * 128],
                                chs[lc][:, dt, :],
                                start=(dt == 0), stop=(dt == DT - 1),
                            )
                    for lc in range(nch):
                        nc.vector.tensor_scalar_add(
                            dst_fn(lc, mt), pss[lc][:, :], b_sb[:, mt:mt + 1],
                        )

            def emit_v_proj(p1pool):
                vtr = VT.rearrange("(t p) s -> p t s", p=128)
                for sc_ in range(S // LCH):
                    ch = spool.tile([128, DT, LCH], BF16, tag="ch",
                                    name=f"vch_{sc_}")
                    nc.sync.dma_start(
                        ch[:, :, :], vtr[:, :, sc_ * LCH:(sc_ + 1) * LCH]
                    )
                    for st4 in range(LCH // 128):
                        st = sc_ * (LCH // 128) + st4
                        ps = p1pool.tile([128, LCH], F32, tag="p1",
                                         name=f"psv_{st}")
                        for dt in range(DT):
                            nc.tensor.matmul(
                                ps[:, 0:M],
                                ch[:, dt, st4 * 128:(st4 + 1) * 128],
                                wv_sb[:, dt, :],
                                start=(dt == 0), stop=(dt == DT - 1),
                            )
                        dstv = v_sb[:, st, :].rearrange(
                            "p (h c) -> p h c", c=65)[:, :, 0:64]
                        srcv = ps[:, 0:M].rearrange("p (h c) -> p h c", c=64)
                        nc.vector.tensor_copy(dstv, srcv)

            def q_dst(lc, mt):
                return qT_sb[:, mt, lc * LCH:(lc + 1) * LCH]

            def k_dst(lc, mt):
                return kT_sb[:, mt, lc * LCH:(lc + 1) * LCH]

            with tc.tile_pool(name="psum1", bufs=p1_bufs, space="PSUM") as p1pool:
                emit_qk_proj(QT, wq_sb, bq_sb, q_dst, L // LCH, p1pool,
                             first=True)
                nc.sync.dma_start(wk_sb[:, :, :],
                                  WK.rearrange("(t p) m -> p t m", p=128))
                nc.sync.dma_start(bk_sb[:, :], BK.rearrange("t p -> p t"))
                emit_qk_proj(KT, wk_sb, bk_sb, k_dst, S // LCH, p1pool)
                nc.sync.dma_start(wv_sb[:, :, :],
                                  WV.rearrange("(t p) m -> p t m", p=128))
                emit_v_proj(p1pool)
            nc.sync.dma_start(we_sb[:, :, :], we_re)

            if debug:
                dbg = {}
                for nm, shp in (
                    ("DBG_WE", [128, 2 * D]), ("DBG_QT", [128, 2 * L]),
                    ("DBG_KT", [128, 2 * S]),
                    ("DBG_V", [128, n_st * HG * 65]),
                ):
                    t = nc.dram_tensor(nm, shp, BF16, kind="ExternalOutput")
                    dbg[nm] = t
                nc.sync.dma_start(
                    dbg["DBG_WE"][:, :],
                    we_sb.rearrange("p a b -> p (a b)"))
                nc.sync.dma_start(
                    dbg["DBG_QT"][:, :],
                    qT_sb.rearrange("p a b -> p (a b)"))
                nc.sync.dma_start(
                    dbg["DBG_KT"][:, :],
                    kT_sb.rearrange("p a b -> p (a b)"))
                nc.sync.dma_start(
                    dbg["DBG_V"][:, :],
                    v_sb.rearrange("p a b -> p (a b)"))

            # ---- phase 2: attention + output projection ----
            with tc.tile_pool(name="psc", bufs=4 // sg, space="PSUM") as scpool, \
                 tc.tile_pool(name="pav", bufs=av_bufs, space="PSUM") as avpool, \
                 tc.tile_pool(name="pfin", bufs=fin_bufs, space="PSUM") as finpool:
                for lc in range(n_lch):
                    if pair_out:
                        outT_sb = opool.tile([128, 2, LCH], PDT, tag="outT")
                    else:
                        outT_sb = opool.tile([64, HG, LCH], PDT, tag="outT")
                    for hpair in range(HG // 2):
                        # two heads' pipelines interleaved so the PE always
                        # has independent work while ACT runs the other
                        # head's exp (two live AV accumulators)
                        hs2 = (hpair * 2, hpair * 2 + 1)
                        avs = {h: avpool.tile([128, LCH], F32, tag="av",
                                              name=f"av_{lc}_{h}")
                               for h in hs2}
                        for g in range(n_sg):
                            # both heads' scores first: the AV matmuls wait
                            # on exp, and the PE queue is in-order -- issuing
                            # av_h0 before sc_h1 would stall sc_h1 (and ACT)
                            # behind exp_h0.
                            exs = {}
                            for h in hs2:
                                p0 = (h % 2) * 64
                                mt = h // 2
                                sc = scpool.tile([128, sg, LCH], F32,
                                                 tag="sc",
                                                 name=f"sc_{lc}_{h}_{g}")
                                for j in range(sg):
                                    st = g * sg + j
                                    nc.tensor.matmul(
                                        sc[:, j, :],
                                        kT_sb[p0:p0 + 64, mt,
                                              st * 128:(st + 1) * 128],
                                        qT_sb[p0:p0 + 64, mt,
                                              lc * LCH:(lc + 1) * LCH],
                                        start=True, stop=True,
                                    )
                                ex = expool.tile([128, sg, LCH], PDT,
                                                 tag="ex",
                                                 name=f"ex_{lc}_{h}_{g}")
                                nc.scalar.activation(
                                    ex[:, :, :], sc[:, :, :], EXP,
                                    bias=0.0, scale=0.125,
                                )
                                exs[h] = ex
                            for h in hs2:
                                av = avs[h]
                                for j in range(sg):
                                    st = g * sg + j
                                    nc.tensor.matmul(
                                        av[0:65, :],
                                        v_sb[:, st, h * 65:(h + 1) * 65],
                                        exs[h][:, j, :],
                                        start=(st == 0),
                                        stop=(st == n_st - 1),
                                    )
                        for h in hs2:
                            av = avs[h]
                            # 1/denominator: ~18-bit fast reciprocal on the
                            # single denominator row (partition 64).
                            rc = npool.tile([65, LCH], F32, tag="rc",
                                            name=f"rc_{lc}_{h}")
                            if recip == "act":
                                # 1/d = exp(-ln d) on ACT; ln and exp share
                                # one ACT table (natural_log_exp_and_others)
                                # so no table reloads.
                                lnd = npool.tile([65, LCH], F32, tag="lnd",
                                                 name=f"lnd_{lc}_{h}")
                                nc.scalar.activation(
                                    lnd[64:65, :], av[64:65, :],
                                    mybir.ActivationFunctionType.Ln)
                                nc.scalar.activation(
                                    rc[64:65, :], lnd[64:65, :],
                                    mybir.ActivationFunctionType.Exp,
                                    bias=0.0, scale=-1.0)
                            elif recip == "fast":
                                nc.vector.reciprocal_approx_fast(
                                    rc[64:65, :], av[64:65, :])
                            else:
                                nc.vector.reciprocal(
                                    rc[64:65, :], av[64:65, :])
                            rep = npool.tile([64, LCH], F32, tag="rep",
                                             name=f"rep_{lc}_{h}")
                            if norm_mode == "bounce":
                                idx = lc * HG + h
                                nc.gpsimd.dma_start(RB[idx:idx + 1, :],
                                                    rc[64:65, :])
                                rb = RB[idx, :]
                                bsrc = AP(rb.tensor, rb.offset,
                                          [[0, 64], [1, LCH]])
                                nc.gpsimd.dma_start(rep[:, :], bsrc)
                            else:
                                nc.gpsimd.partition_broadcast(
                                    rep[:, :], rc[64:65, :])
                            if not pair_out:
                                nc.vector.tensor_mul(
                                    outT_sb[:, h, :], av[0:64, :], rep[:, :],
                                )
                            elif h % 2 == 0:
                                nc.vector.tensor_mul(
                                    outT_sb[0:64, h // 2, :],
                                    av[0:64, :], rep[:, :],
                                )
                            else:
                                # engines can't shift partition bases;
                                # stage in 0-63 and DMA to 64-127.
                                tmp = npool.tile([64, LCH], PDT, tag="tmp",
                                                 name=f"tmp_{lc}_{h}")
                                nc.vector.tensor_mul(
                                    tmp[:, :], av[0:64, :], rep[:, :],
                                )
                                nc.gpsimd.dma_start(
                                    outT_sb[64:128, h // 2, :], tmp[:, :])
                    for ls in range(LCH // 128):
                        ob = obpool.tile([128, D], F32, tag="ob")
                        if pair_out:
                            pfs = [finpool.tile([128, 512], F32, tag="fin",
                                                name=f"fin_{lc}_{ls}_{dmc}")
                                   for dmc in range(2)]
                            for hp in range(2):
                                for dmc in range(2):
                                    nc.tensor.matmul(
                                        pfs[dmc][:, :],
                                        outT_sb[:, hp,
                                                ls * 128:(ls + 1) * 128],
                                        we_sb[:, hp,
                                              dmc * 512:(dmc + 1) * 512],
                                        start=(hp == 0), stop=(hp == 1),
                                    )
                            for dmc in range(2):
                                nc.vector.tensor_copy(
                                    ob[:, dmc * 512:(dmc + 1) * 512],
                                    pfs[dmc][:, :]
                                )
                        else:
                            for dmc in range(2):
                                pf = finpool.tile([128, 512], F32, tag="fin")
                                for h in range(HG):
                                    nc.tensor.matmul(
                                        pf[:, :],
                                        outT_sb[:, h, ls * 128:(ls + 1) * 128],
                                        we_sb[:, h,
                                              dmc * 512:(dmc + 1) * 512],
                                        start=(h == 0), stop=(h == HG - 1),
                                    )
                                nc.vector.tensor_copy(
                                    ob[:, dmc * 512:(dmc + 1) * 512], pf[:, :]
                                )
                        row0 = lc * LCH + ls * 128
                        nc.sync.dma_start(OUT[row0:row0 + 128, :], ob[:, :])

    nc.compile()
    return nc


def make_in_maps(Q, K, V, Wq, bq, Wk, bk, Wv, We, p2dt="bf16"):
    """Per-core input dicts. Core c: batch c//4, head-group c%4."""
    from ml_dtypes import bfloat16

    qt = [np.ascontiguousarray(Q[b].T).astype(bfloat16) for b in range(B)]
    kt = [np.ascontiguousarray(K[b].T).astype(bfloat16) for b in range(B)]
    vt = [np.ascontiguousarray(V[b].T).astype(bfloat16) for b in range(B)]
    in_maps = []
    for c in range(N_CORES):
        b = c // 4
        g = c % 4
        cs = slice(g * M, (g + 1) * M)
        in_maps.append({
            "QT": qt[b], "KT": kt[b], "VT": vt[b],
            "WQ": np.ascontiguousarray(Wq[:, cs]).astype(bfloat16),
            "WK": np.ascontiguousarray(Wk[:, cs]).astype(bfloat16),
            "WV": np.ascontiguousarray(Wv[:, cs]).astype(bfloat16),
            "WE": (np.ascontiguousarray(We[cs, :]).astype(bfloat16)
                   if p2dt == "bf16" else
                   np.ascontiguousarray(We[cs, :])),
            "BQ": np.ascontiguousarray(bq[cs]).reshape(2, 128),
            "BK": np.ascontiguousarray(bk[cs]).reshape(2, 128),
        })
    return in_maps


_NC_CACHE = {}


def run(Q, K, V, Wq, bq, Wk, bk, Wv, bv, We, be, trace=False, **build_kw):
    from concourse.bass_utils import run_bass_kernel_spmd

    L = Q.shape[1]
    key = (L, tuple(sorted(build_kw.items())))
    if key not in _NC_CACHE:
        _NC_CACHE[key] = build_program(L=L, S=K.shape[1], **build_kw)
    nc = _NC_CACHE[key]
    in_maps = make_in_maps(Q, K, V, Wq, bq, Wk, bk, Wv, We,
                           p2dt=build_kw.get('p2dt', 'bf16'))
    res = run_bass_kernel_spmd(
        nc, in_maps, core_ids=list(range(N_CORES)), trace=trace
    )
    out = np.zeros((B, L, D), np.float32)
    for c in range(N_CORES):
        out[c // 4] += np.asarray(res.results[c]["OUT"], np.float32)
    # softmax rows sum to 1 => +bv shifts every attention row by bv;
    # be is a plain output shift.
    out += (bv.astype(np.float64) @ We.astype(np.float64) + be).astype(np.float32)
    return out, res


def kernel(Q, K, V, Wq, bq, Wk, bk, Wv, bv, We, be):
    args = [np.asarray(x, np.float32) for x in
            (Q, K, V, Wq, bq, Wk, bk, Wv, bv, We, be)]
    out, _ = run(*args)
    return out



# revision 14
# speedup vs baseline: 1.0148x; 1.0148x over previous
"""Trainium2 Bass kernel for nn_AttentionLayer (B=2, L=S=2048, D=1024, H=16).

Sharding: batch x head-group. Core c handles batch b=c//4 and heads
[4*(c%4), 4*(c%4)+4). Column-parallel Wq/Wk/Wv, row-parallel We; the
per-core partial outputs are summed on the host (4 partials per batch).

Key design points (v2):
  - PE p-state: the tensor engine only reaches 2.4 GHz after ~3us of
    gap-free execution (1.2 GHz otherwise). Phase 2 is software-pipelined
    so the PE never waits on exp: AV matmuls for score-group g-1 are
    issued after the scores of group g, by which time the exp of g-1 has
    long finished on ACT/Pool.
  - exp split: host folds 0.125*log2(e) into Wq, so softmax weights are
    2^scores. ACT computes exp(ln2*x) for most tiles; a fraction is
    routed as DVE copy (PSUM->SBUF) + GpSimd pow(2, x) (Pool cannot read
    PSUM), keeping ACT below the PE roofline.
  - denominator: ones-column in the V tile (softmax denominator rides the
    AV matmul for free); reciprocal via DVE reciprocal_approx_fast on the
    single denominator row; DRAM-bounce DMA broadcasts it across 64
    partitions on the gpsimd queue.
  - output projection for l-chunk lc is deferred into lc+1's score loop
    (hides the normalize chain) and its PSUM result is DMA'd straight to
    DRAM (no DVE copy).
  - phase 1 runs weight-stationary with 1024-wide l-chunks so every DMA
    line is 2KB.

Host folds the zero-cost pieces: bv and be shift every output row by
(bv @ We + be) because softmax rows sum to 1; bq/bk are applied
on-device (per-partition scalar adds).
"""
import sys

for _p in ("/opt/trn_rl_repo", "/root/.axon_site/_ro/trn_rl_repo"):
    if _p not in sys.path:
        sys.path.insert(0, _p)

import numpy as np

import concourse.bass as bass
import concourse.mybir as mybir
from concourse import bacc
from concourse.bass import AP
from concourse.tile import TileContext

F32 = mybir.dt.float32
BF16 = mybir.dt.bfloat16

D = 1024          # model dim
H_TOTAL = 16
HG = 4            # heads per core
E = 64            # head dim
M = HG * E        # 256 projected cols per core
DT = D // 128     # 8 d-tiles
LCH = 512         # phase-2 l-chunk
P1CH = 1024       # phase-1 l-chunk (2KB DMA lines)
B = 2
N_CORES = 8
LN2 = 0.6931471805599453
LOG2E = 1.4426950408889634


def build_program(L=2048, S=2048, sg=2, stream_bufs=6, ex_bufs=4, cp_bufs=3,
                  n_bufs=3, o_bufs=2, p1_bufs=3, av_bufs=2, sc_bufs=2,
                  pool_mod=4, pool_rem=(), oproj_hp=0, oproj_g=4,
                  p2dt="bf16", debug=0, recip="fast"):
    nc = bacc.Bacc("TRN2")
    PDT = BF16 if p2dt == "bf16" else F32
    QT = nc.dram_tensor("QT", [D, L], BF16, kind="ExternalInput")
    KT = nc.dram_tensor("KT", [D, S], BF16, kind="ExternalInput")
    VT = nc.dram_tensor("VT", [D, S], BF16, kind="ExternalInput")
    WQ = nc.dram_tensor("WQ", [D, M], BF16, kind="ExternalInput")
    WK = nc.dram_tensor("WK", [D, M], BF16, kind="ExternalInput")
    WV = nc.dram_tensor("WV", [D, M], BF16, kind="ExternalInput")
    WE = nc.dram_tensor("WE", [M, D], BF16, kind="ExternalInput")
    BQ = nc.dram_tensor("BQ", [2, 128], F32, kind="ExternalInput")
    BK = nc.dram_tensor("BK", [2, 128], F32, kind="ExternalInput")
    OUT = nc.dram_tensor("OUT", [L, D], F32, kind="ExternalOutput")
    RB = nc.dram_tensor("RB", [(L // LCH) * HG, LCH], F32, kind="Internal")

    n_lch = L // LCH
    n_st = S // 128
    n_sg = n_st // sg
    EXP = mybir.ActivationFunctionType.Exp

    with TileContext(nc) as tc:
        with tc.tile_pool(name="const", bufs=1) as cpool, \
             tc.tile_pool(name="stream", bufs=stream_bufs) as spool, \
             tc.tile_pool(name="ex", bufs=ex_bufs) as expool, \
             tc.tile_pool(name="cp", bufs=cp_bufs) as cppool, \
             tc.tile_pool(name="norm", bufs=n_bufs) as npool, \
             tc.tile_pool(name="ob", bufs=2) as obpool, \
             tc.tile_pool(name="outw", bufs=o_bufs) as opool:

            wq_sb = cpool.tile([128, DT, M], BF16, tag="wq")
            wk_sb = cpool.tile([128, DT, M], BF16, tag="wk")
            wv_sb = cpool.tile([128, DT, M], BF16, tag="wv")
            we_sb = cpool.tile([128, 2, D], BF16, tag="we")
            we_re = WE.rearrange("(h e) n -> e h n", e=128)
            bq_sb = cpool.tile([128, 2], F32, tag="bq")
            bk_sb = cpool.tile([128, 2], F32, tag="bk")
            nc.sync.dma_start(wq_sb[:, :, :],
                              WQ.rearrange("(t p) m -> p t m", p=128))
            nc.sync.dma_start(bq_sb[:, :], BQ.rearrange("t p -> p t"))

            qT_sb = cpool.tile([128, 2, L], PDT, tag="qT")
            kT_sb = cpool.tile([128, 2, S], PDT, tag="kT")
            v_sb = cpool.tile([128, n_st, HG * 65], PDT, tag="v")
            two_sb = cpool.tile([128, sg, LCH], F32, tag="two")
            # ones-column per head (denominator trick): pre-fill everything
            # with 1.0; the projection copies overwrite the 64 value columns
            # of each head, leaving column 64 as the ones column.
            if p2dt == "bf16":
                nc.vector.memset(v_sb[:, :, :], 1.0)
            else:
                nc.vector.memset(v_sb[:, :, :].bitcast(F32), 1.0)
            nc.vector.memset(two_sb[:, :, :], 2.0)

            # ---- phase 1: projections ----
            def emit_qk_proj(XT, w_sb, b_sb, dst_fn, nch, p1pool):
                # weight-stationary: all chunks resident, loop (mt, dt)
                # outer and chunks inner so each stationary is loaded once
                # per nch moving matmuls.
                xtr = XT.rearrange("(t p) l -> p t l", p=128)
                chs = []
                for c in range(nch):
                    ch = spool.tile([128, DT, P1CH], BF16, tag="ch",
                                    name=f"ch_{c}")
                    nc.sync.dma_start(
                        ch[:, :, :], xtr[:, :, c * P1CH:(c + 1) * P1CH]
                    )
                    chs.append(ch)
                for mt in range(2):
                    pss = [p1pool.tile([128, P1CH], F32, tag="p1",
                                       name=f"ps_{mt}_{c}")
                           for c in range(nch)]
                    for dt in range(DT):
                        for c in range(nch):
                            # matmul output must fit one PSUM bank
                            # (512 f32) -> two half-matmuls per chunk
                            for hf in range(P1CH // 512):
                                nc.tensor.matmul(
                                    pss[c][:, hf * 512:(hf + 1) * 512],
                                    w_sb[:, dt, mt * 128:(mt + 1) * 128],
                                    chs[c][:, dt, hf * 512:(hf + 1) * 512],
                                    start=(dt == 0), stop=(dt == DT - 1),
                                )
                    for c in range(nch):
                        nc.vector.tensor_scalar_add(
                            dst_fn(c, mt), pss[c][:, :], b_sb[:, mt:mt + 1],
                        )

            def emit_v_proj(p1pool):
                vtr = VT.rearrange("(t p) s -> p t s", p=128)
                for sc_ in range(S // P1CH):
                    ch = spool.tile([128, DT, P1CH], BF16, tag="ch",
                                    name=f"vch_{sc_}")
                    nc.sync.dma_start(
                        ch[:, :, :], vtr[:, :, sc_ * P1CH:(sc_ + 1) * P1CH]
                    )
                    for st4 in range(P1CH // 128):
                        st = sc_ * (P1CH // 128) + st4
                        ps = p1pool.tile([128, M], F32, tag="pv",
                                         name=f"psv_{st}", bufs=2)
                        for dt in range(DT):
                            nc.tensor.matmul(
                                ps[:, :],
                                ch[:, dt, st4 * 128:(st4 + 1) * 128],
                                wv_sb[:, dt, :],
                                start=(dt == 0), stop=(dt == DT - 1),
                            )
                        dstv = v_sb[:, st, :].rearrange(
                            "p (h c) -> p h c", c=65)[:, :, 0:64]
                        srcv = ps[:, :].rearrange("p (h c) -> p h c", c=64)
                        nc.vector.tensor_copy(dstv, srcv)

            def q_dst(c, mt):
                return qT_sb[:, mt, c * P1CH:(c + 1) * P1CH]

            def k_dst(c, mt):
                return kT_sb[:, mt, c * P1CH:(c + 1) * P1CH]

            with tc.tile_pool(name="psum1", bufs=p1_bufs, space="PSUM") \
                    as p1pool:
                emit_qk_proj(QT, wq_sb, bq_sb, q_dst, L // P1CH, p1pool)
                nc.sync.dma_start(wk_sb[:, :, :],
                                  WK.rearrange("(t p) m -> p t m", p=128))
                nc.sync.dma_start(bk_sb[:, :], BK.rearrange("t p -> p t"))
                emit_qk_proj(KT, wk_sb, bk_sb, k_dst, S // P1CH, p1pool)
                nc.sync.dma_start(wv_sb[:, :, :],
                                  WV.rearrange("(t p) m -> p t m", p=128))
                emit_v_proj(p1pool)
            nc.sync.dma_start(we_sb[:, :, :], we_re)

            if debug:
                for nm, t in (("DBG_QT", qT_sb), ("DBG_KT", kT_sb),
                              ("DBG_V", v_sb)):
                    shp = [128, int(np.prod(t.shape[1:]))]
                    dt_ = nc.dram_tensor(nm, shp, PDT,
                                         kind="ExternalOutput")
                    nc.sync.dma_start(dt_[:, :],
                                      t.rearrange("p a b -> p (a b)"))

            # ---- phase 2: attention + output projection ----
            with tc.tile_pool(name="psc", bufs=sc_bufs, space="PSUM") \
                    as scpool, \
                 tc.tile_pool(name="pav", bufs=av_bufs, space="PSUM") \
                    as avpool, \
                 tc.tile_pool(name="pfin", bufs=1, space="PSUM") as finpool:

                outTs = {}

                def emit_out_proj(lc):
                    outT_sb = outTs.pop(lc)
                    for ls in range(LCH // 128):
                        pfs = [finpool.tile([128, 512], F32,
                                            tag=f"fin{dmc}",
                                            name=f"fin_{lc}_{ls}_{dmc}")
                               for dmc in range(2)]
                        for hp in range(2):
                            for dmc in range(2):
                                nc.tensor.matmul(
                                    pfs[dmc][:, :],
                                    outT_sb[:, hp, ls * 128:(ls + 1) * 128],
                                    we_sb[:, hp, dmc * 512:(dmc + 1) * 512],
                                    start=(hp == 0), stop=(hp == 1),
                                )
                        ob = obpool.tile([128, D], F32, tag="ob",
                                         name=f"ob_{lc}_{ls}")
                        row0 = lc * LCH + ls * 128
                        for dmc in range(2):
                            nc.vector.tensor_copy(
                                ob[:, dmc * 512:(dmc + 1) * 512],
                                pfs[dmc][:, :])
                        nc.sync.dma_start(OUT[row0:row0 + 128, :], ob[:, :])

                def emit_normalize(lc, h, av, outT_sb):
                    # Denominator row (PSUM partition 64) -> SBUF -> DRAM
                    # bounce broadcasts it across partitions 0-63; the
                    # reciprocal then runs at partition base 0
                    # (reciprocal_approx_fast is broken for PSUM inputs and
                    # nonzero partition bases).
                    idx = lc * HG + h
                    dn = npool.tile([65, LCH], F32, tag="dn",
                                    name=f"dn_{lc}_{h}")
                    nc.vector.tensor_copy(dn[64:65, :], av[64:65, :])
                    nc.gpsimd.dma_start(RB[idx:idx + 1, :], dn[64:65, :])
                    rb = RB[idx, :]
                    bsrc = AP(rb.tensor, rb.offset, [[0, 64], [1, LCH]])
                    rep = npool.tile([64, LCH], F32, tag="rep",
                                     name=f"rep_{lc}_{h}")
                    nc.gpsimd.dma_start(rep[:, :], bsrc)
                    rc = npool.tile([64, LCH], F32, tag="rc",
                                    name=f"rc_{lc}_{h}")
                    if recip == "fast":
                        nc.vector.reciprocal_approx_fast(
                            rc[:, :], rep[:, :])
                    else:
                        nc.vector.reciprocal(rc[:, :], rep[:, :])
                    if h % 2 == 0:
                        nc.vector.tensor_mul(
                            outT_sb[0:64, h // 2, :], av[0:64, :], rc[:, :],
                        )
                    else:
                        # engines can't shift partition bases; stage in
                        # 0-63 and DMA to 64-127.
                        tmp = npool.tile([64, LCH], PDT, tag="tmp",
                                         name=f"tmp_{lc}_{h}")
                        nc.vector.tensor_mul(
                            tmp[:, :], av[0:64, :], rc[:, :],
                        )
                        nc.gpsimd.dma_start(
                            outT_sb[64:128, h // 2, :], tmp[:, :])

                for lc in range(n_lch):
                    outT_sb = opool.tile([128, 2, LCH], PDT, tag="outT",
                                         name=f"outT_{lc}")
                    outTs[lc] = outT_sb
                    lcs = slice(lc * LCH, (lc + 1) * LCH)
                    for hpair in range(HG // 2):
                        hs2 = (hpair * 2, hpair * 2 + 1)
                        avs = {h: avpool.tile([128, LCH], F32, tag="av",
                                              name=f"av_{lc}_{h}")
                               for h in hs2}

                        def emit_av(g, exs):
                            for h in hs2:
                                for j in range(sg):
                                    st = g * sg + j
                                    nc.tensor.matmul(
                                        avs[h][0:65, :],
                                        v_sb[:, st, h * 65:(h + 1) * 65],
                                        exs[h][:, j, :],
                                        start=(st == 0),
                                        stop=(st == n_st - 1),
                                    )

                        prev_exs = None
                        for g in range(n_sg):
                            exs = {}
                            for hi, h in enumerate(hs2):
                                p0 = (h % 2) * 64
                                mt = h // 2
                                sc = scpool.tile([128, sg, LCH], F32,
                                                 tag="sc",
                                                 name=f"sc_{lc}_{h}_{g}")
                                for j in range(sg):
                                    st = g * sg + j
                                    nc.tensor.matmul(
                                        sc[:, j, :],
                                        kT_sb[p0:p0 + 64, mt,
                                              st * 128:(st + 1) * 128],
                                        qT_sb[p0:p0 + 64, mt, lcs],
                                        start=True, stop=True,
                                    )
                                ex = expool.tile([128, sg, LCH], PDT,
                                                 tag="ex",
                                                 name=f"ex_{lc}_{h}_{g}")
                                if (g * 2 + hi) % pool_mod in pool_rem:
                                    # DVE copy to SBUF, then 2^x on the
                                    # Pool engine (it cannot read PSUM).
                                    cp = cppool.tile([128, sg, LCH], F32,
                                                     tag="cp",
                                                     name=f"cp_{lc}_{h}_{g}")
                                    nc.vector.tensor_copy(
                                        cp[:, :, :], sc[:, :, :])
                                    nc.gpsimd.tensor_tensor(
                                        ex[:, :, :], two_sb[:, :, :],
                                        cp[:, :, :],
                                        op=mybir.AluOpType.pow)
                                else:
                                    nc.scalar.activation(
                                        ex[:, :, :], sc[:, :, :], EXP,
                                        bias=0.0, scale=LN2,
                                    )
                                exs[h] = ex
                            if g > 0:
                                emit_av(g - 1, prev_exs)
                            if (lc > 0 and hpair == oproj_hp
                                    and g == oproj_g):
                                emit_out_proj(lc - 1)
                            prev_exs = exs
                        emit_av(n_sg - 1, prev_exs)
                        for h in hs2:
                            emit_normalize(lc, h, avs[h], outT_sb)
                emit_out_proj(n_lch - 1)

    nc.compile()
    return nc


def make_in_maps(Q, K, V, Wq, bq, Wk, bk, Wv, We):
    """Per-core input dicts. Core c: batch c//4, head-group c%4.

    0.125*log2(e) is folded into Wq/bq so softmax weights are 2^scores.
    """
    from ml_dtypes import bfloat16

    f = 0.125 * LOG2E
    Wq = Wq * f
    bq = bq * f
    qt = [np.ascontiguousarray(Q[b].T).astype(bfloat16) for b in range(B)]
    kt = [np.ascontiguousarray(K[b].T).astype(bfloat16) for b in range(B)]
    vt = [np.ascontiguousarray(V[b].T).astype(bfloat16) for b in range(B)]
    in_maps = []
    for c in range(N_CORES):
        b = c // 4
        g = c % 4
        cs = slice(g * M, (g + 1) * M)
        in_maps.append({
            "QT": qt[b], "KT": kt[b], "VT": vt[b],
            "WQ": np.ascontiguousarray(Wq[:, cs]).astype(bfloat16),
            "WK": np.ascontiguousarray(Wk[:, cs]).astype(bfloat16),
            "WV": np.ascontiguousarray(Wv[:, cs]).astype(bfloat16),
            "WE": np.ascontiguousarray(We[cs, :]).astype(bfloat16),
            "BQ": np.ascontiguousarray(bq[cs]).reshape(2, 128),
            "BK": np.ascontiguousarray(bk[cs]).reshape(2, 128),
        })
    return in_maps


_NC_CACHE = {}


def run(Q, K, V, Wq, bq, Wk, bk, Wv, bv, We, be, trace=False, **build_kw):
    from concourse.bass_utils import run_bass_kernel_spmd

    L = Q.shape[1]
    key = (L, tuple(sorted(build_kw.items())))
    if key not in _NC_CACHE:
        _NC_CACHE[key] = build_program(L=L, S=K.shape[1], **build_kw)
    nc = _NC_CACHE[key]
    in_maps = make_in_maps(Q, K, V, Wq, bq, Wk, bk, Wv, We)
    res = run_bass_kernel_spmd(
        nc, in_maps, core_ids=list(range(N_CORES)), trace=trace
    )
    out = np.zeros((B, L, D), np.float32)
    for c in range(N_CORES):
        out[c // 4] += np.asarray(res.results[c]["OUT"], np.float32)
    # softmax rows sum to 1 => +bv shifts every attention row by bv;
    # be is a plain output shift.
    out += (bv.astype(np.float64) @ We.astype(np.float64)
            + be).astype(np.float32)
    return out, res


def kernel(Q, K, V, Wq, bq, Wk, bk, Wv, bv, We, be):
    args = [np.asarray(x, np.float32) for x in
            (Q, K, V, Wq, bq, Wk, bk, Wv, bv, We, be)]
    out, _ = run(*args)
    return out
